# revision 4
# baseline (speedup 1.0000x reference)
"""nn_ComplexNetAttention on 8 trn2 NeuronCores.

Sharding: heads column-parallel for QKV+attention (2 heads/core), one
AllToAll to redistribute attention output from head-sharded to
token-sharded (+ per-token absmax rows piggybacked), token-parallel
o-projection (256 tokens/core). All matmuls fp16: integer-quantized
activations are exact in fp16, weights fp16 (~2^-11 rel).
"""
import numpy as np
import ml_dtypes

import concourse.bass as bass
import concourse.bacc as bacc
import concourse.tile as tile
import concourse.mybir as mybir
from concourse.bass_utils import run_bass_kernel_spmd

f32 = mybir.dt.float32
f16 = mybir.dt.float16
bf16 = mybir.dt.bfloat16

T, H, NH, D = 2048, 2048, 16, 128
NC = 8
HPC = NH // NC          # heads per core = 2
DS = HPC * D            # d_out slice per core = 256
TS = T // NC            # tokens per core for o-proj = 256
ROWS = 2 * DS + 2       # A2A shard rows: out_r^T | out_i^T | maxr | maxi = 514
MAGIC = float(2**23 + 2**22)  # fp32 round-to-nearest-even integer trick
A_OP = mybir.AluOpType
HT = H // 128           # 16


def build_nc():
    nc = bacc.Bacc("TRN2", target_bir_lowering=False, debug=False, num_devices=NC)
    A = {}
    def inp(name, shape, dt=f32):
        A[name] = nc.dram_tensor(name, shape, dt, kind="ExternalInput").ap()
    inp("xrT", [H, T]); inp("xiT", [H, T])
    inp("srow_r", [1, T]); inp("srow_i", [1, T])
    inp("invrow_r", [1, T]); inp("invrow_i", [1, T])
    inp("invcol_r", [T, 1]); inp("invcol_i", [T, 1])
    for t_ in ("q", "k"):
        inp(f"w{t_}_rT", [H, DS], f16); inp(f"w{t_}_iT", [H, DS], f16)
    inp("wv1", [H, 2 * DS], f16); inp("wv2", [H, 2 * DS], f16)
    inp("wo_rT", [H, H], f16); inp("wo_iT", [H, H], f16)
    inp("cosT", [D, T], f16); inp("sinT", [D, T], f16)
    inp("masks", [128, 4 * 512], f16)
    inp("ident", [128, 128], f32)
    A["yr_part"] = nc.dram_tensor("yr_part", [TS, H], f32, kind="ExternalOutput").ap()
    A["yi_part"] = nc.dram_tensor("yi_part", [TS, H], f32, kind="ExternalOutput").ap()
    return nc, A


def _chunked(ap):
    """DRAM [H, w] -> [128, HT, w] view (partition, h-chunk, col)."""
    return ap.rearrange("(a b) c -> b a c", b=128)


def emit(nc, A, tc, ctx):
    const = ctx.enter_context(tc.tile_pool(name="const", bufs=1))
    ps = ctx.enter_context(tc.tile_pool(name="ps", bufs=1, space="PSUM"))
    dram = ctx.enter_context(tc.tile_pool(name="dram", bufs=1, space="DRAM"))

    ident = const.tile([128, 128], f32, name="ident_t")
    nc.sync.dma_start(ident[:], A["ident"][:])
    masks = const.tile([128, 4 * 512], f16, name="masks_t")
    nc.sync.dma_start(masks[:], A["masks"][:])
    cosT = const.tile([D, T], f16, name="cosT_t")
    nc.sync.dma_start(cosT[:], A["cosT"][:])
    sinT = const.tile([D, T], f16, name="sinT_t")
    nc.sync.dma_start(sinT[:], A["sinT"][:])
    inv_rep_r = const.tile([128, T], f32, name="invrep_r")
    nc.sync.dma_start(inv_rep_r[:], A["invrow_r"][:].to_broadcast((128, T)))
    inv_rep_i = const.tile([128, T], f32, name="invrep_i")
    nc.sync.dma_start(inv_rep_i[:], A["invrow_i"][:].to_broadcast((128, T)))
    invc_r = const.tile([128, 16, 1], f32, name="invc_r")
    nc.sync.dma_start(invc_r[:], A["invcol_r"][:].rearrange("(a b) c -> b a c", b=128))
    invc_i = const.tile([128, 16, 1], f32, name="invc_i")
    nc.sync.dma_start(invc_i[:], A["invcol_i"][:].rearrange("(a b) c -> b a c", b=128))

    _pn = [0]
    def psum(tag, shape=(128, 512)):
        _pn[0] += 1
        return ps.tile(list(shape), f32, name=f"pt{_pn[0]}", tag=tag)

    cont = dram.tile([NC * ROWS, TS], f32, name="cont")
    ag2 = dram.tile([NC * ROWS, TS], f32, name="ag2")

    with tc.tile_pool(name="qk", bufs=1) as qk_pool, \
         tc.tile_pool(name="vj", bufs=1) as vj_pool:
        qrot = {}
        for tn in ("q", "k"):
            for hd in range(HPC):
                for cp in ("r", "i"):
                    qrot[(tn, hd, cp)] = qk_pool.tile([128, T], f16, name=f"{tn}rot{hd}{cp}")
        vjoin = {}
        for hd in range(HPC):
            for bk in range(T // 128):
                vjoin[(hd, bk)] = vj_pool.tile([128, 257], bf16, name=f"vj{hd}_{bk}")
                nc.vector.memset(vjoin[(hd, bk)][:, 256:257], 1.0)
        wv1 = vj_pool.tile([128, HT, 512], f16, name="wv1_t")
        nc.sync.dma_start(wv1[:], _chunked(A["wv1"][:]))
        wv2 = vj_pool.tile([128, HT, 512], f16, name="wv2_t")
        nc.sync.dma_start(wv2[:], _chunked(A["wv2"][:]))

        # ======== phase 1+2: quant + projections, in t-halves ========
        for half in range(4):
            t0 = half * (T // 4)
            hw = T // 4
            with tc.tile_pool(name=f"acts{half}", bufs=1) as acts_pool, \
                 tc.tile_pool(name=f"xq{half}", bufs=2) as xq, \
                 tc.tile_pool(name=f"wq{half}", bufs=2) as wq:
                acts = {}
                for cp in ("r", "i"):
                    srep = xq.tile([128, hw], f32, name=f"srep{cp}{half}", tag=f"srep{cp}")
                    nc.sync.dma_start(srep[:], A[f"srow_{cp}"][:, t0:t0 + hw].to_broadcast((128, hw)))
                    for ht in range(HT):
                        xt = xq.tile([128, hw], f32, name=f"xt{cp}{ht}{half}", tag="xt")
                        nc.sync.dma_start(xt[:], A[f"x{cp}T"][ht * 128:(ht + 1) * 128, t0:t0 + hw])
                        nc.vector.tensor_tensor(xt[:], xt[:], srep[:], A_OP.mult)
                        a = acts_pool.tile([128, hw], f16, name=f"acts{cp}{ht}_{half}")
                        nc.vector.tensor_scalar(a[:], xt[:], MAGIC, MAGIC, A_OP.add, A_OP.subtract)
                        acts[(cp, ht)] = a

                for tn in ("q", "k"):
                    for dt_ in range(HPC):
                        ds0 = dt_ * 128
                        wr = wq.tile([128, HT, 128], f16, name=f"wr{tn}{dt_}{half}", tag="wr")
                        wi = wq.tile([128, HT, 128], f16, name=f"wi{tn}{dt_}{half}", tag="wi")
                        nc.sync.dma_start(wr[:], _chunked(A[f"w{tn}_rT"][:, ds0:ds0 + 128]))
                        nc.sync.dma_start(wi[:], _chunked(A[f"w{tn}_iT"][:, ds0:ds0 + 128]))
                        pg = {"A": psum(f"p{dt_*4}"), "B": psum(f"p{dt_*4+1}"),
                              "C": psum(f"p{dt_*4+2}"), "D": psum(f"p{dt_*4+3}")}
                        pA = [pg["A"]]; pB = [pg["B"]]; pC = [pg["C"]]; pD = [pg["D"]]
                        for h in range(HT):
                            st = (h == 0); sp = (h == HT - 1)
                            for grp, wt, ckey in ((pA, wr, "r"), (pB, wi, "i"),
                                                  (pC, wi, "r"), (pD, wr, "i")):
                                nc.tensor.matmul(grp[0][:], wt[:, h, :], acts[(ckey, h)][:, 0:512],
                                                 start=st, stop=sp)
                        for tb in range(1):
                            gsl = slice(t0, t0 + 512)
                            t1 = xq.tile([128, 512], f32, name=f"t1{tn}{dt_}{tb}{half}", tag="dr1")
                            t2 = xq.tile([128, 512], f32, name=f"t2{tn}{dt_}{tb}{half}", tag="dr2")
                            y_r = xq.tile([128, 512], f32, name=f"ydr{tn}{dt_}{tb}{half}", tag="dr3")
                            y_i = xq.tile([128, 512], f32, name=f"ydi{tn}{dt_}{tb}{half}", tag="dr4")
                            nc.vector.tensor_tensor(t1[:], pA[tb][:], inv_rep_r[:, gsl], A_OP.mult)
                            nc.vector.tensor_tensor(t2[:], pB[tb][:], inv_rep_i[:, gsl], A_OP.mult)
                            nc.vector.tensor_tensor(y_r[:], t1[:], t2[:], A_OP.add)
                            nc.vector.tensor_tensor(t1[:], pC[tb][:], inv_rep_r[:, gsl], A_OP.mult)
                            nc.vector.tensor_tensor(t2[:], pD[tb][:], inv_rep_i[:, gsl], A_OP.mult)
                            nc.vector.tensor_tensor(y_i[:], t1[:], t2[:], A_OP.subtract)
                            nc.vector.tensor_tensor(t1[:], y_r[:], cosT[:, gsl], A_OP.mult)
                            nc.vector.tensor_tensor(t2[:], y_i[:], sinT[:, gsl], A_OP.mult)
                            nc.vector.tensor_tensor(qrot[(tn, dt_, "r")][:, gsl], t1[:], t2[:],
                                                    A_OP.subtract)
                            nc.vector.tensor_tensor(t1[:], y_i[:], cosT[:, gsl], A_OP.mult)
                            nc.vector.tensor_tensor(t2[:], y_r[:], sinT[:, gsl], A_OP.mult)
                            nc.vector.tensor_tensor(qrot[(tn, dt_, "i")][:, gsl], t1[:], t2[:],
                                                    A_OP.add)

                for tt in range(hw // 128):
                    bk = half * (hw // 128) + tt
                    pAv = psum("p0"); pBv = psum("p1")
                    for h in range(HT):
                        st = (h == 0); sp = (h == HT - 1)
                        nc.tensor.matmul(pAv[:], acts[("r", h)][:, tt * 128:tt * 128 + 128],
                                         wv1[:, h, :], start=st, stop=sp)
                        nc.tensor.matmul(pBv[:], acts[("i", h)][:, tt * 128:tt * 128 + 128],
                                         wv2[:, h, :], start=st, stop=sp)
                    for hd in range(HPC):
                        for ci in range(2):
                            sl = slice(ci * 256 + hd * 128, ci * 256 + hd * 128 + 128)
                            tv = xq.tile([128, 128], f32, name=f"tv{bk}{hd}{ci}", tag="tv")
                            nc.vector.tensor_scalar(tv[:], pAv[:, sl], invc_r[:, bk, :],
                                                    None, A_OP.mult)
                            nc.vector.scalar_tensor_tensor(
                                vjoin[(hd, bk)][:, ci * 128:ci * 128 + 128],
                                pBv[:, sl], invc_i[:, bk, :], tv[:], A_OP.mult, A_OP.add)

        # ======== phase 3: attention per head ========
        SC = float(1.0 / np.sqrt(2 * D))
        with tc.tile_pool(name="attn", bufs=1) as at, \
             tc.tile_pool(name="epool", bufs=2) as ep, \
             tc.tile_pool(name="tp", bufs=2) as tp:
            out_nat = {}
            pmax = {}
            for hd in range(HPC):
                for cp in ("r", "i"):
                    for bq in range(T // 128):
                        out_nat[(hd, cp, bq)] = at.tile([128, 128], f32, name=f"on{hd}{cp}{bq}")
            for hd in range(HPC):
                for g in range(4):
                    etiles = {}
                    for bk in range(4 * g + 4):
                        pS = psum(f"p{bk % 4}")
                        qsl = slice(g * 512, g * 512 + 512)
                        nc.tensor.matmul(pS[:], qrot[("k", hd, "r")][:, bk * 128:bk * 128 + 128],
                                         qrot[("q", hd, "r")][:, qsl], start=True, stop=False)
                        nc.tensor.matmul(pS[:], qrot[("k", hd, "i")][:, bk * 128:bk * 128 + 128],
                                         qrot[("q", hd, "i")][:, qsl], start=False, stop=True)
                        if bk >= 4 * g:
                            mc = (bk - 4 * g) * 512
                            nc.vector.tensor_tensor(pS[:], pS[:], masks[:, mc:mc + 512], A_OP.add)
                        e = ep.tile([128, 512], bf16, name=f"e{hd}{g}_{bk}", tag=f"e{bk}")
                        nc.scalar.activation(e[:], pS[:], mybir.ActivationFunctionType.Exp, scale=SC)
                        etiles[bk] = e
                    for bq in range(4 * g, 4 * g + 4):
                        pO = psum(f"p{4 + bq % 4}", (128, 257))
                        col = (bq - 4 * g) * 128
                        for bk in range(bq + 1):
                            nc.tensor.matmul(pO[:], etiles[bk][:, col:col + 128], vjoin[(hd, bk)][:],
                                             start=(bk == 0), stop=(bk == bq))
                        rec = at.tile([128, 1], f32, name=f"rec{hd}{bq}", tag="rec")
                        nc.vector.reciprocal(rec[:], pO[:, 256:257])
                        for ci, cp in enumerate(("r", "i")):
                            o = out_nat[(hd, cp, bq)]
                            nc.vector.tensor_scalar(o[:], pO[:, ci * 128:ci * 128 + 128], rec[:],
                                                    None, A_OP.mult)
                            if hd == 0:
                                pmax[(cp, bq)] = at.tile([128, 1], f32, name=f"mx{cp}{bq}")
                                nc.vector.tensor_reduce(pmax[(cp, bq)][:], o[:], mybir.AxisListType.X,
                                                        A_OP.max, apply_absolute_value=True)
                            else:
                                mx = at.tile([128, 1], f32, name=f"mxt{hd}{cp}{bq}", tag="mxt")
                                nc.vector.tensor_reduce(mx[:], o[:], mybir.AxisListType.X,
                                                        A_OP.max, apply_absolute_value=True)
                                nc.vector.tensor_tensor(pmax[(cp, bq)][:], pmax[(cp, bq)][:], mx[:],
                                                        A_OP.max)

            # ======== phase 4: transpose + assemble + AllToAll ========
            for ci, cp in enumerate(("r", "i")):
                for hd in range(HPC):
                    oT = tp.tile([128, T], f32, name=f"oT{cp}{hd}", tag="oT")
                    for bq in range(T // 128):
                        pT = psum(f"p{bq % 2}", (128, 128))
                        nc.tensor.transpose(pT[:], out_nat[(hd, cp, bq)][:], ident[:])
                        nc.vector.tensor_copy(oT[:, bq * 128:bq * 128 + 128], pT[:])
                    r0 = ci * DS + hd * 128
                    for s in range(NC):
                        nc.sync.dma_start(cont[s * ROWS + r0: s * ROWS + r0 + 128, :],
                                          oT[:, s * TS:(s + 1) * TS])
                for bq in range(T // 128):
                    s, c0 = bq // 2, (bq % 2) * 128
                    dst = cont[s * ROWS + 2 * DS + ci: s * ROWS + 2 * DS + ci + 1, c0:c0 + 128]
                    nc.sync.dma_start(dst.rearrange("a b -> b a"), pmax[(cp, bq)][:])
            nc.gpsimd.collective_compute(
                "AllToAll", A_OP.bypass, replica_groups=[list(range(NC))],
                ins=[cont[:].opt()], outs=[ag2[:].opt()])

    # ======== phase 5: o-projection on my 256-token slice ========
    with tc.tile_pool(name="op", bufs=1) as op, \
         tc.tile_pool(name="wo", bufs=1) as wo, \
         tc.tile_pool(name="od", bufs=2) as od:
        agrows = ag2[:].rearrange("(s r) c -> r s c", r=ROWS)   # [514, 8, 256]
        gmax = {}
        for ci, cp in enumerate(("r", "i")):
            mrows = op.tile([1, NC, TS], f32, name=f"mrows{cp}")
            nc.sync.dma_start(mrows[:], agrows[2 * DS + ci: 2 * DS + ci + 1, :, :])
            g = op.tile([1, TS], f32, name=f"gmax{cp}")
            nc.vector.tensor_tensor(g[:], mrows[:, 0, :], mrows[:, 1, :], A_OP.max)
            for s in range(2, NC):
                nc.vector.tensor_tensor(g[:], g[:], mrows[:, s, :], A_OP.max)
            nc.vector.tensor_scalar(g[:], g[:], 1e-5, None, A_OP.max)
            gmax[cp] = g
        bounce = dram.tile([4, TS], f32, name="bounce")
        srep_o = {}; invcol_o = {}
        for ci, cp in enumerate(("r", "i")):
            rg = op.tile([1, TS], f32, name=f"rg{cp}")
            nc.vector.reciprocal(rg[:], gmax[cp][:])
            nc.vector.tensor_scalar(rg[:], rg[:], 127.0, None, A_OP.mult)
            nc.sync.dma_start(bounce[ci:ci + 1, :], rg[:])
            iv = op.tile([1, TS], f32, name=f"iv{cp}")
            nc.vector.tensor_scalar(iv[:], gmax[cp][:], float(1.0 / 127.0), None, A_OP.mult)
            nc.sync.dma_start(bounce[2 + ci:3 + ci, :], iv[:])
            sr = op.tile([128, TS], f32, name=f"srepo{cp}")
            nc.sync.dma_start(sr[:], bounce[ci:ci + 1, :].to_broadcast((128, TS)))
            srep_o[cp] = sr
            ic = op.tile([128, 2, 1], f32, name=f"invco{cp}")
            nc.sync.dma_start(ic[:], bounce[2 + ci:3 + ci, :].rearrange("c (a b) -> b a c", b=128))
            invcol_o[cp] = ic
        ninvcol_i = op.tile([128, 2, 1], f32, name="ninvcoi")
        nc.vector.tensor_scalar(ninvcol_i[:], invcol_o["i"][:], -1.0, None, A_OP.mult)

        xo = {}
        for ci, cp in enumerate(("r", "i")):
            for c2 in range(NC):
                for dd in range(2):
                    r0 = c2 * ROWS + ci * DS + dd * 128
                    xt = op.tile([128, TS], f32, name=f"xof{cp}{c2}{dd}", tag="xof")
                    nc.sync.dma_start(xt[:], ag2[r0:r0 + 128, :])
                    m = op.tile([128, TS], f32, name=f"xom{cp}{c2}{dd}", tag="xom")
                    nc.vector.tensor_tensor(m[:], xt[:], srep_o[cp][:], A_OP.mult)
                    a = op.tile([128, TS], f16, name=f"xoi{cp}{c2}{dd}")
                    nc.vector.tensor_scalar(a[:], m[:], MAGIC, MAGIC, A_OP.add, A_OP.subtract)
                    xo[(cp, c2 * 2 + dd)] = a

        for jb in range(4):
            pA = [psum("p0"), psum("p1")]
            pB = [psum("p2"), psum("p3")]
            pC = [psum("p4"), psum("p5")]
            pD = [psum("p6"), psum("p7")]
            wr = wo.tile([128, HT, 512], f16, name=f"wor{jb}", tag="wor")
            wi = wo.tile([128, HT, 512], f16, name=f"woi{jb}", tag="woi")
            nc.sync.dma_start(wr[:], _chunked(A["wo_rT"][:, jb * 512:(jb + 1) * 512]))
            nc.sync.dma_start(wi[:], _chunked(A["wo_iT"][:, jb * 512:(jb + 1) * 512]))
            for d16 in range(16):
                st = (d16 == 0); sp = (d16 == 15)
                for tt in range(2):
                    lr = xo[("r", d16)][:, tt * 128:(tt + 1) * 128]
                    li = xo[("i", d16)][:, tt * 128:(tt + 1) * 128]
                    nc.tensor.matmul(pA[tt][:], lr, wr[:, d16, :], start=st, stop=sp)
                    nc.tensor.matmul(pC[tt][:], lr, wi[:, d16, :], start=st, stop=sp)
                    nc.tensor.matmul(pB[tt][:], li, wi[:, d16, :], start=st, stop=sp)
                    nc.tensor.matmul(pD[tt][:], li, wr[:, d16, :], start=st, stop=sp)
            for tt in range(2):
                fr = od.tile([128, 512], f32, name=f"fr{jb}{tt}", tag="fr")
                t1 = od.tile([128, 512], f32, name=f"ft{jb}{tt}", tag="ft")
                nc.vector.tensor_scalar(t1[:], pA[tt][:], invcol_o["r"][:, tt, :], None, A_OP.mult)
                nc.vector.scalar_tensor_tensor(fr[:], pB[tt][:], invcol_o["i"][:, tt, :], t1[:],
                                               A_OP.mult, A_OP.add)
                nc.sync.dma_start(A["yr_part"][tt * 128:(tt + 1) * 128, jb * 512:(jb + 1) * 512], fr[:])
                fi = od.tile([128, 512], f32, name=f"fi{jb}{tt}", tag="fi")
                t2 = od.tile([128, 512], f32, name=f"ft2{jb}{tt}", tag="ft2")
                nc.vector.tensor_scalar(t2[:], pC[tt][:], invcol_o["r"][:, tt, :], None, A_OP.mult)
                nc.vector.scalar_tensor_tensor(fi[:], pD[tt][:], ninvcol_i[:, tt, :], t2[:],
                                               A_OP.mult, A_OP.add)
                nc.sync.dma_start(A["yi_part"][tt * 128:(tt + 1) * 128, jb * 512:(jb + 1) * 512], fi[:])


_CACHE = {}

def _get_compiled():
    if "nc" not in _CACHE:
        from contextlib import ExitStack
        nc, A = build_nc()
        with tile.TileContext(nc) as tc:
            with ExitStack() as ctx:
                emit(nc, A, tc, ctx)
        nc.compile()
        _CACHE["nc"] = nc
    return _CACHE["nc"]


def _host_prep(hidden_real, hidden_imag, positions,
               Wq_r, Wq_i, Wk_r, Wk_i, Wv_r, Wv_i, Wo_r, Wo_i):
    fp16 = np.float16
    f = np.float32
    hr = np.asarray(hidden_real, f); hi = np.asarray(hidden_imag, f)
    m_r = np.maximum(np.abs(hr).max(1), f(1e-5)).astype(f)
    m_i = np.maximum(np.abs(hi).max(1), f(1e-5)).astype(f)
    s_r = (f(127.0) / m_r).astype(f); s_i = (f(127.0) / m_i).astype(f)
    inv_r = (f(1.0) / s_r).astype(f); inv_i = (f(1.0) / s_i).astype(f)
    inv_freq = (f(1.0) / (f(10000.0) ** (np.arange(D, dtype=f) / f(D)))).astype(f)
    freqs = np.asarray(positions, np.int32).astype(f)[:, None] * inv_freq[None, :]
    cos = np.cos(freqs).astype(f).astype(ml_dtypes.bfloat16).astype(fp16)
    sin = np.sin(freqs).astype(f).astype(ml_dtypes.bfloat16).astype(fp16)
    col = np.arange(512)[None, :]; row = np.arange(128)[:, None]
    masks = np.concatenate(
        [np.where(col >= 128 * m + row, f(0.0), f(-60000.0)) for m in range(4)],
        axis=1).astype(fp16)
    base = {
        "xrT": np.ascontiguousarray(hr.T), "xiT": np.ascontiguousarray(hi.T),
        "srow_r": s_r[None, :], "srow_i": s_i[None, :],
        "invrow_r": inv_r[None, :], "invrow_i": inv_i[None, :],
        "invcol_r": np.ascontiguousarray(inv_r[:, None]),
        "invcol_i": np.ascontiguousarray(inv_i[:, None]),
        "wo_rT": np.ascontiguousarray(np.asarray(Wo_r, f).T.astype(fp16)),
        "wo_iT": np.ascontiguousarray(np.asarray(Wo_i, f).T.astype(fp16)),
        "cosT": np.ascontiguousarray(cos.T), "sinT": np.ascontiguousarray(sin.T),
        "masks": masks, "ident": np.eye(128, dtype=f),
    }
    in_maps = []
    for c in range(NC):
        sl = slice(c * DS, (c + 1) * DS)
        im = dict(base)
        for nm, Wr_, Wi_ in (("q", Wq_r, Wq_i), ("k", Wk_r, Wk_i)):
            im[f"w{nm}_rT"] = np.ascontiguousarray(np.asarray(Wr_, f)[sl].T.astype(fp16))
            im[f"w{nm}_iT"] = np.ascontiguousarray(np.asarray(Wi_, f)[sl].T.astype(fp16))
        vr = np.asarray(Wv_r, f)[sl].T.astype(fp16)
        vi = np.asarray(Wv_i, f)[sl].T.astype(fp16)
        im["wv1"] = np.ascontiguousarray(np.concatenate([vr, vi], axis=1))
        im["wv2"] = np.ascontiguousarray(np.concatenate([vi, -vr], axis=1))
        in_maps.append(im)
    return in_maps


def kernel(**inputs):
    nc = _get_compiled()
    in_maps = _host_prep(**inputs)
    res = run_bass_kernel_spmd(nc, in_maps, list(range(NC)))
    yr = np.concatenate([res.results[c]["yr_part"] for c in range(NC)], axis=0)
    yi = np.concatenate([res.results[c]["yi_part"] for c in range(NC)], axis=0)
    return yr, yi

